# revision 12
# baseline (speedup 1.0000x reference)
"""Distributed Trainium2 kernel for nn_CategoricalDecoder (topk_masking).

Math (identical to the reference, algebraically simplified):
  logits = relu(z @ W1 + b1) @ W2 + b2                      # [NZ, D]
  log_prob_bins = x @ log_sig.T + (1-x) @ log_1m.T + log_w
                = x @ logits.T + v + log_w                  # log_sig - log_1m = logits
    where v[n] = sum_d log_1m[n, d],  log_1m = log_sigmoid(-logits)
  top-k selection by S = x@logits.T + v + log_w; the re-decode of the selected
  bins recomputes exactly T = S - log_w, so out = logsumexp(top16 T) - log 16.

Distribution: NZ=16384 sharded 8 ways (2048 bins/core); x and weights
replicated.  Per core: decode shard -> S,T slices -> local top-16 of S
(max8 x2) -> AllGather 16 candidates/core -> per-core identical theta
(16th largest of 128) and M (max + 16) -> masked exp partial sums ->
AllReduce -> log.  Every core computes the identical final [512] output.
"""

import math
from contextlib import ExitStack

import numpy as np
import ml_dtypes

import concourse.bass as bass
import concourse.mybir as mybir
import concourse.tile as tile
from concourse import bacc
from concourse.bass_utils import run_bass_kernel_spmd
from concourse import bass_isa
from concourse.bass import _add_dep_helper

bf16 = ml_dtypes.bfloat16
f8e3 = ml_dtypes.float8_e4m3

NCORES = 8
B, NZ, LAT, HID, D = 512, 16384, 64, 512, 1536
SH = NZ // NCORES      # 2048 bins per core
K = 16
NB = 4                 # column blocks per shard
NBS = SH // NB         # 512
BT = B // 128          # 4 row tiles
DT = D // 128          # 12
KT = HID // 128        # 4
NEG = -1.0e30

# module-level knobs for the local test harness (graded contract is kernel(**inputs))
TRACE = False
LAST_RESULT = None

_BUILT = None


def _body(tc, ctx, x_d, z_d, w1_d, w2_d, b1_d, b2_d, lw_d, out_d):
    nc = tc.nc
    f32 = mybir.dt.float32
    bf = mybir.dt.bfloat16
    f8 = mybir.dt.float8e4
    AF = mybir.ActivationFunctionType
    OP = mybir.AluOpType

    singles = ctx.enter_context(tc.tile_pool(name="singles", bufs=1))
    hpool = ctx.enter_context(tc.tile_pool(name="hpool", bufs=2))
    lpool = ctx.enter_context(tc.tile_pool(name="lpool", bufs=2))
    sgpool = ctx.enter_context(tc.tile_pool(name="sgpool", bufs=1))
    l1mpool = ctx.enter_context(tc.tile_pool(name="l1mpool", bufs=2))
    accpool = ctx.enter_context(tc.tile_pool(name="accpool", bufs=2))
    spool = ctx.enter_context(tc.tile_pool(name="spool", bufs=2))
    epool = ctx.enter_context(tc.tile_pool(name="epool", bufs=2))
    cpool = ctx.enter_context(tc.tile_pool(name="cpool", bufs=4))
    ps_l1 = ctx.enter_context(tc.tile_pool(name="ps_l1", bufs=2, space="PSUM"))
    ps_l2 = ctx.enter_context(tc.tile_pool(name="ps_l2", bufs=3, space="PSUM"))
    ps_sc = ctx.enter_context(tc.tile_pool(name="ps_sc", bufs=2, space="PSUM"))
    ps_p = ctx.enter_context(tc.tile_pool(name="ps_p", bufs=1, space="PSUM"))
    dram = ctx.enter_context(tc.tile_pool(name="dram", bufs=1, space="DRAM"))

    # persistent SBUF
    w1_sb = singles.tile([LAT, HID], bf)          # [64, 512]
    w2_sb = singles.tile([128, HID // 256, 2, D], f8)   # (kp, c, j, m), W2*32
    zT_sb = singles.tile([LAT, SH], bf)           # [64, 2048]
    xT_sb = singles.tile([128, D // 256, 2, B], f8)     # (dp, c, j, b)
    b1_sb = singles.tile([128, KT], f32)
    negb2_sb = singles.tile([128, DT], f32)
    b2_sb = singles.tile([128, DT], f32)
    lw_bc = singles.tile([128, SH], f32)
    ones_bf = singles.tile([128, 1], bf)
    v_bc = singles.tile([128, SH], f32)           # v = sum_d log_1m, bcast on parts
    w_bc = singles.tile([128, SH], f32)           # w = v + log_w
    T_sb = singles.tile([128, BT, SH], f32)       # T_raw = G (x @ logits.T), no v
    cand64 = singles.tile([128, BT, 4 * K], f32)  # per-block top16 candidates
    TH = singles.tile([128, BT], f32)             # theta per (p, bt)
    negM = singles.tile([128, BT], f32)           # -(global maxS+16)
    nmloc = singles.tile([128, BT], f32)          # -(local maxS+16)
    fac = singles.tile([128, BT], f32)            # exp(Mloc - M)
    E_sb = singles.tile([128, BT, SH], bf)        # exp(T - Mloc), bf16
    Ssum = singles.tile([128, BT], f32)
    sum_sb = singles.tile([128, BT], f32)
    ln_sb = singles.tile([128, BT], f32)
    out_sb = singles.tile([128, BT], f32)

    cand_l = dram.tile([B, K], f32)
    cand_a = dram.tile([NCORES, B, K], f32)
    part_l = dram.tile([B], f32)
    part_s = dram.tile([B], f32)

    # ---- loads ----
    nc.vector.memset(ones_bf, 1.0)
    nc.sync.dma_start(out=w1_sb, in_=w1_d)
    nc.sync.dma_start(out=zT_sb, in_=z_d)
    nc.sync.dma_start(out=b1_sb, in_=b1_d.rearrange("(t p) -> p t", p=128))
    nc.sync.dma_start(out=b2_sb, in_=b2_d.rearrange("(t p) -> p t", p=128))
    nc.sync.dma_start(out=w2_sb, in_=w2_d)
    nc.sync.dma_start(out=xT_sb, in_=x_d)
    nc.vector.tensor_scalar_mul(negb2_sb, b2_sb, -1.0)
    lw_bcast_ap = bass.AP(
        tensor=lw_d.tensor, offset=lw_d.offset, ap=[[0, 128]] + list(lw_d.ap)
    )
    nc.sync.dma_start(out=lw_bc, in_=lw_bcast_ap)

    # ---- decode + score, block by block ----
    prev_last_ln = None
    for nb in range(NB):
        ncol = slice(nb * NBS, (nb + 1) * NBS)
        # L1: h.T = relu(W1.T @ z.T + b1)  -> [512, 512] bf16 as 4 ptiles
        hT = hpool.tile([128, HID // 256, 2, NBS], f8)
        for ht in range(KT):
            p1 = ps_l1.tile([128, NBS], f32)
            nc.tensor.matmul(
                p1, lhsT=w1_sb[:, ht * 128:(ht + 1) * 128], rhs=zT_sb[:, ncol],
                start=True, stop=True,
            )
            nc.scalar.activation(
                out=hT[:, ht // 2, ht % 2, :], in_=p1, func=AF.Relu,
                bias=b1_sb[:, ht:ht + 1],
            )
        # L2: logits.T = W2.T @ h.T + b2  -> [1536, 512] bf16 as 12 ptiles
        # sg = sigmoid(-logits) in f32; log_1m = Ln(sg) batched afterwards so
        # the ScalarE LUT table loads twice per block, not per instruction.
        lT = lpool.tile([128, DT // 2, 2, NBS], f8)
        sg = sgpool.tile([128, DT, NBS], f32)
        sig_insts = []
        for dt in range(DT):
            p2 = ps_l2.tile([128, NBS], f32)
            for c in range(HID // 256):
                nc.tensor.matmul(
                    p2,
                    lhsT=w2_sb[:, c, :, dt * 128:(dt + 1) * 128],
                    rhs=hT[:, c, :, :],
                    start=(c == 0), stop=(c == HID // 256 - 1),
                    perf_mode=mybir.MatmulPerfMode.DoubleRow,
                )
            # psum = 32 * (h @ W2); logits = psum/32 + b2  (on DVE: ScalarE is
            # the main-loop bottleneck, Vector has slack)
            nc.vector.tensor_scalar(
                lT[:, dt // 2, dt % 2, :], p2, 1.0 / 32.0, b2_sb[:, dt:dt + 1],
                op0=OP.mult, op1=OP.add,
            )
            si = nc.scalar.activation(
                out=sg[:, dt, :], in_=p2, func=AF.Sigmoid,
                scale=-1.0 / 32.0, bias=negb2_sb[:, dt:dt + 1],
            )
            if prev_last_ln is not None:
                _add_dep_helper(si.ins, prev_last_ln.ins,
                                reason="act-table phase: sig after prev ln batch")
            sig_insts.append(si)
        # Ln batch (3 tiles per op); v = sum_d log_1m via PE ones-matmul
        l1m = l1mpool.tile([128, DT, NBS], bf, tag="l1m")
        last_ln = None
        for dg in range(DT // 3):
            li = nc.scalar.activation(
                out=l1m[:, dg * 3:(dg + 1) * 3, :],
                in_=sg[:, dg * 3:(dg + 1) * 3, :], func=AF.Ln
            )
            _add_dep_helper(li.ins, sig_insts[-1].ins,
                            reason="act-table phase: ln after sig batch")
            last_ln = li
        prev_last_ln = last_ln
        pp = ps_p.tile([1, NBS], f32)
        for dt in range(DT):
            nc.tensor.matmul(
                pp, lhsT=ones_bf, rhs=l1m[:, dt, :],
                start=(dt == 0), stop=(dt == DT - 1),
            )
        v_row = accpool.tile([1, NBS], f32, tag="v_row")
        nc.vector.tensor_copy(v_row, pp)
        nc.gpsimd.partition_broadcast(v_bc[:, ncol], v_row)
        nc.vector.tensor_add(w_bc[:, ncol], v_bc[:, ncol], lw_bc[:, ncol])
        # scoring: G = x @ logits.T -> T_raw ; S = w + T_raw ; per-block top16
        for bt in range(BT):
            p3 = ps_sc.tile([128, NBS], f32)
            for c in range(D // 256):
                nc.tensor.matmul(
                    p3,
                    lhsT=xT_sb[:, c, :, bt * 128:(bt + 1) * 128],
                    rhs=lT[:, c, :, :],
                    start=(c == 0), stop=(c == D // 256 - 1),
                    perf_mode=mybir.MatmulPerfMode.DoubleRow,
                )
            # store S = w + G directly; scratch copy for the destructive top-k
            nc.vector.tensor_add(T_sb[:, bt, ncol], w_bc[:, ncol], p3)
            Sx = spool.tile([128, NBS], f32, tag="Sx")
            nc.gpsimd.tensor_copy(Sx, T_sb[:, bt, ncol])
            cs = slice(nb * K, nb * K + 8)
            cs2 = slice(nb * K + 8, (nb + 1) * K)
            nc.vector.max(cand64[:, bt, cs], Sx)
            nc.vector.match_replace(
                out=Sx, in_to_replace=cand64[:, bt, cs], in_values=Sx,
                imm_value=NEG,
            )
            nc.vector.max(cand64[:, bt, cs2], Sx)

    # ---- merge per-block candidates -> local top-16 of S per row ----
    cand_dmas = []
    for bt in range(BT):
        c1 = cpool.tile([128, 8], f32, tag="c1")
        c2 = cpool.tile([128, 8], f32, tag="c2")
        nc.vector.max(c1, cand64[:, bt, :])
        nc.vector.match_replace(
            out=cand64[:, bt, :], in_to_replace=c1, in_values=cand64[:, bt, :],
            imm_value=NEG,
        )
        nc.vector.max(c2, cand64[:, bt, :])
        d1 = nc.sync.dma_start(out=cand_l[bt * 128:(bt + 1) * 128, 0:8], in_=c1)
        d2 = nc.sync.dma_start(out=cand_l[bt * 128:(bt + 1) * 128, 8:16], in_=c2)
        cand_dmas.extend([d1, d2])
        # local shifted max: -(local maxS + 16); exact rescale post-gather
        nc.vector.tensor_scalar(
            nmloc[:, bt:bt + 1], c1[:, 0:1], 16.0, -1.0, op0=OP.add, op1=OP.mult
        )

    nc.gpsimd.collective_compute(
        "AllGather", mybir.AluOpType.bypass,
        replica_groups=[list(range(NCORES))],
        ins=[cand_l.opt()], outs=[cand_a.opt()],
    )

    # ---- theta (16th of 128) and M (max + 16), identical on every core ----
    for bt in range(BT):
        la = cpool.tile([128, NCORES * K], f32, tag="la")
        nc.sync.dma_start(
            out=la.rearrange("p (c s) -> p c s", s=K),
            in_=cand_a[:, bt * 128:(bt + 1) * 128, :].rearrange("c p s -> p c s"),
        )
        t8a = cpool.tile([128, 8], f32, tag="t8a")
        t8b = cpool.tile([128, 8], f32, tag="t8b")
        nc.vector.max(t8a, la)
        nc.vector.tensor_scalar(
            negM[:, bt:bt + 1], t8a[:, 0:1], 16.0, -1.0, op0=OP.add, op1=OP.mult
        )
        nc.vector.match_replace(
            out=la, in_to_replace=t8a, in_values=la, imm_value=NEG
        )
        nc.vector.max(t8b, la)
        nc.vector.tensor_copy(TH[:, bt:bt + 1], t8b[:, 7:8])

    # ---- masked exp partial sums ----
    # Everything except the theta-select happens BEFORE/DURING the AllGather:
    # S = w + T_raw in place (bit-identical op to the candidate computation),
    # tv = T = S - lw, and E = exp(T - Mloc) against the LOCAL shifted max.
    # Post-gather only: esel = (S >= theta) * E, then partials are rescaled by
    # fac = exp(Mloc - M) so every core's sum is against the global M.
    for bt in range(BT):
        tv = epool.tile([128, SH], f32, tag="tv")
        ti = nc.vector.tensor_sub(tv, T_sb[:, bt, :], lw_bc)
        for dd in cand_dmas:
            _add_dep_helper(ti.ins, dd.ins,
                            reason="start collective before tail precompute")
        nc.scalar.activation(
            out=E_sb[:, bt, :], in_=tv, func=AF.Exp, bias=nmloc[:, bt:bt + 1]
        )
    for bt in range(BT):
        sel = epool.tile([128, SH], f32, tag="sel")
        nc.vector.scalar_tensor_tensor(
            out=sel, in0=T_sb[:, bt, :], scalar=TH[:, bt:bt + 1], in1=E_sb[:, bt, :],
            op0=OP.is_ge, op1=OP.mult,
            accum_out=Ssum[:, bt:bt + 1],
        )
        # fac = exp(Mloc - M) = exp((-nmloc) + negM ... ) computed from tiny APs
        nc.vector.tensor_sub(fac[:, bt:bt + 1], negM[:, bt:bt + 1],
                             nmloc[:, bt:bt + 1])
    nc.scalar.activation(out=fac, in_=fac, func=AF.Exp)
    nc.vector.tensor_mul(Ssum, Ssum, fac)

    nc.sync.dma_start(out=part_l.rearrange("(p t) -> p t", t=BT), in_=Ssum)
    nc.gpsimd.collective_compute(
        "AllReduce", mybir.AluOpType.add,
        replica_groups=[list(range(NCORES))],
        ins=[part_l.opt()], outs=[part_s.opt()],
    )
    nc.sync.dma_start(out=sum_sb, in_=part_s.rearrange("(p t) -> p t", t=BT))
    nc.scalar.activation(out=ln_sb, in_=sum_sb, func=AF.Ln)
    nc.vector.tensor_scalar_add(ln_sb, ln_sb, -math.log(float(K)))
    nc.vector.tensor_sub(out_sb, ln_sb, negM)  # + M
    nc.sync.dma_start(out=out_d.rearrange("(t p) -> p t", p=128), in_=out_sb)


def _build():
    f32 = mybir.dt.float32
    bf = mybir.dt.bfloat16
    f8 = mybir.dt.float8e4
    nc = bacc.Bacc(
        "TRN2", target_bir_lowering=False, debug=False, num_devices=NCORES
    )
    x_d = nc.dram_tensor("x", [128, D // 256, 2, B], f8, kind="ExternalInput").ap()
    z_d = nc.dram_tensor("z", [LAT, SH], bf, kind="ExternalInput").ap()
    w1_d = nc.dram_tensor("w1", [LAT, HID], bf, kind="ExternalInput").ap()
    w2_d = nc.dram_tensor("w2", [128, HID // 256, 2, D], f8, kind="ExternalInput").ap()
    b1_d = nc.dram_tensor("b1", [HID], f32, kind="ExternalInput").ap()
    b2_d = nc.dram_tensor("b2", [D], f32, kind="ExternalInput").ap()
    lw_d = nc.dram_tensor("lw", [SH], f32, kind="ExternalInput").ap()
    out_d = nc.dram_tensor("out", [B], f32, kind="ExternalOutput").ap()

    with tile.TileContext(nc) as tc:
        with ExitStack() as ctx:
            _body(tc, ctx, x_d, z_d, w1_d, w2_d, b1_d, b2_d, lw_d, out_d)
    nc.compile()
    return nc


def _get_built():
    global _BUILT
    if _BUILT is None:
        _BUILT = _build()
    return _BUILT


def make_in_maps(x, log_w, z, W1, b1, W2, b2):
    x = np.asarray(x, np.float32)
    log_w = np.asarray(log_w, np.float32)
    z = np.asarray(z, np.float32)
    # x.T packed for fp8 DoubleRow: [dp, c, j, b], d = 256c + 128j + dp
    xb = np.ascontiguousarray(
        x.astype(f8e3).T.reshape(6, 2, 128, B).transpose(2, 0, 1, 3))
    w1b = np.ascontiguousarray(np.asarray(W1, np.float32).astype(bf16))
    # (W2 * 32) packed for fp8 DoubleRow over K=HID: [kp, c, j, m]
    w2b = np.ascontiguousarray(
        (np.asarray(W2, np.float32) * 32.0).astype(f8e3)
        .reshape(2, 2, 128, D).transpose(2, 0, 1, 3))
    b1f = np.ascontiguousarray(np.asarray(b1, np.float32))
    b2f = np.ascontiguousarray(np.asarray(b2, np.float32))
    in_maps = []
    for c in range(NCORES):
        zs = np.ascontiguousarray(z[c * SH:(c + 1) * SH].astype(bf16).T)  # [LAT, SH]
        lws = np.ascontiguousarray(log_w[c * SH:(c + 1) * SH])
        in_maps.append(
            {"x": xb, "z": zs, "w1": w1b, "w2": w2b, "b1": b1f, "b2": b2f,
             "lw": lws}
        )
    return in_maps


def kernel(x, log_w, z, W1, b1, W2, b2, k, n_chunks):
    global LAST_RESULT
    assert int(k) == K, f"kernel compiled for k={K}, got {k}"
    nc = _get_built()
    in_maps = make_in_maps(x, log_w, z, W1, b1, W2, b2)
    res = run_bass_kernel_spmd(
        nc, in_maps, core_ids=list(range(NCORES)), trace=TRACE
    )
    LAST_RESULT = res
    return np.asarray(res.results[0]["out"], np.float32).reshape(B)


# revision 14
# speedup vs baseline: 1.0045x; 1.0045x over previous
"""Distributed Trainium2 kernel for nn_CategoricalDecoder (topk_masking).

Math (identical to the reference, algebraically simplified):
  logits = relu(z @ W1 + b1) @ W2 + b2                      # [NZ, D]
  log_prob_bins = x @ log_sig.T + (1-x) @ log_1m.T + log_w
                = x @ logits.T + v + log_w                  # log_sig - log_1m = logits
    where v[n] = sum_d log_1m[n, d],  log_1m = log_sigmoid(-logits)
  top-k selection by S = x@logits.T + v + log_w; the re-decode of the selected
  bins recomputes exactly T = S - log_w, so out = logsumexp(top16 T) - log 16.

Distribution: NZ=16384 sharded 8 ways (2048 bins/core); x and weights
replicated.  Per core: decode shard -> S,T slices -> local top-16 of S
(max8 x2) -> AllGather 16 candidates/core -> per-core identical theta
(16th largest of 128) and M (max + 16) -> masked exp partial sums ->
AllReduce -> log.  Every core computes the identical final [512] output.
"""

import math
from contextlib import ExitStack

import numpy as np
import ml_dtypes

import concourse.bass as bass
import concourse.mybir as mybir
import concourse.tile as tile
from concourse import bacc
from concourse.bass_utils import run_bass_kernel_spmd
from concourse import bass_isa
from concourse.bass import _add_dep_helper

bf16 = ml_dtypes.bfloat16
f8e3 = ml_dtypes.float8_e4m3

NCORES = 8
B, NZ, LAT, HID, D = 512, 16384, 64, 512, 1536
SH = NZ // NCORES      # 2048 bins per core
K = 16
NB = 4                 # column blocks per shard
NBS = SH // NB         # 512
BT = B // 128          # 4 row tiles
DT = D // 128          # 12
KT = HID // 128        # 4
NEG = -1.0e30

# module-level knobs for the local test harness (graded contract is kernel(**inputs))
TRACE = False
LAST_RESULT = None

_BUILT = None


def _body(tc, ctx, x_d, z_d, w1_d, w2_d, b1_d, b2_d, lw_d, out_d):
    nc = tc.nc
    f32 = mybir.dt.float32
    bf = mybir.dt.bfloat16
    f8 = mybir.dt.float8e4
    AF = mybir.ActivationFunctionType
    OP = mybir.AluOpType

    singles = ctx.enter_context(tc.tile_pool(name="singles", bufs=1))
    hpool = ctx.enter_context(tc.tile_pool(name="hpool", bufs=2))
    lpool = ctx.enter_context(tc.tile_pool(name="lpool", bufs=2))
    sgpool = ctx.enter_context(tc.tile_pool(name="sgpool", bufs=2))
    l1mpool = ctx.enter_context(tc.tile_pool(name="l1mpool", bufs=2))
    accpool = ctx.enter_context(tc.tile_pool(name="accpool", bufs=2))
    spool = ctx.enter_context(tc.tile_pool(name="spool", bufs=2))
    epool = ctx.enter_context(tc.tile_pool(name="epool", bufs=2))
    cpool = ctx.enter_context(tc.tile_pool(name="cpool", bufs=4))
    ps_l1 = ctx.enter_context(tc.tile_pool(name="ps_l1", bufs=1, space="PSUM"))
    ps_l2 = ctx.enter_context(tc.tile_pool(name="ps_l2", bufs=4, space="PSUM"))
    ps_sc = ctx.enter_context(tc.tile_pool(name="ps_sc", bufs=2, space="PSUM"))
    ps_p = ctx.enter_context(tc.tile_pool(name="ps_p", bufs=1, space="PSUM"))
    dram = ctx.enter_context(tc.tile_pool(name="dram", bufs=1, space="DRAM"))

    # persistent SBUF
    w1_sb = singles.tile([LAT, HID], bf)          # [64, 512]
    w2_sb = singles.tile([128, HID // 256, 2, D], f8)   # (kp, c, j, m), W2*32
    zT_sb = singles.tile([LAT, SH], bf)           # [64, 2048]
    xT_sb = singles.tile([128, D // 256, 2, B], f8)     # (dp, c, j, b)
    b1_sb = singles.tile([128, KT], f32)
    b2_sb = singles.tile([128, DT], f32)
    lw_bc = singles.tile([128, SH], f32)
    ones_bf = singles.tile([128, 1], bf)
    v_bc = singles.tile([128, SH], f32)           # v = sum_d log_1m, bcast on parts
    w_bc = singles.tile([128, SH], f32)           # w = v + log_w
    T_sb = singles.tile([128, BT, SH], f32)       # T_raw = G (x @ logits.T), no v
    cand64 = singles.tile([128, BT, 4 * K], f32)  # per-block top16 candidates
    TH = singles.tile([128, BT], f32)             # theta per (p, bt)
    negM = singles.tile([128, BT], f32)           # -(global maxS+16)
    nmloc = singles.tile([128, BT], f32)          # -(local maxS+16)
    fac = singles.tile([128, BT], f32)            # exp(Mloc - M)
    E_sb = singles.tile([128, BT, SH], bf)        # exp(T - Mloc), bf16
    Ssum = singles.tile([128, BT], f32)
    sum_sb = singles.tile([128, BT], f32)
    ln_sb = singles.tile([128, BT], f32)
    out_sb = singles.tile([128, BT], f32)

    cand_l = dram.tile([B, K], f32)
    cand_a = dram.tile([NCORES, B, K], f32)
    part_l = dram.tile([B], f32)
    part_s = dram.tile([B], f32)

    # ---- loads ----
    nc.vector.memset(ones_bf, 1.0)
    nc.sync.dma_start(out=w1_sb, in_=w1_d)
    nc.sync.dma_start(out=zT_sb, in_=z_d)
    nc.sync.dma_start(out=b1_sb, in_=b1_d.rearrange("(t p) -> p t", p=128))
    nc.sync.dma_start(out=b2_sb, in_=b2_d.rearrange("(t p) -> p t", p=128))
    nc.sync.dma_start(out=w2_sb, in_=w2_d)
    nc.sync.dma_start(out=xT_sb, in_=x_d)
    lw_bcast_ap = bass.AP(
        tensor=lw_d.tensor, offset=lw_d.offset, ap=[[0, 128]] + list(lw_d.ap)
    )
    nc.sync.dma_start(out=lw_bc, in_=lw_bcast_ap)

    # ---- decode + score, block by block ----
    for nb in range(NB):
        ncol = slice(nb * NBS, (nb + 1) * NBS)
        # L1: h.T = relu(W1.T @ z.T + b1)  -> [512, 512] bf16 as 4 ptiles
        hT = hpool.tile([128, HID // 256, 2, NBS], f8)
        for ht in range(KT):
            p1 = ps_l1.tile([128, NBS], f32)
            nc.tensor.matmul(
                p1, lhsT=w1_sb[:, ht * 128:(ht + 1) * 128], rhs=zT_sb[:, ncol],
                start=True, stop=True,
            )
            nc.scalar.activation(
                out=hT[:, ht // 2, ht % 2, :], in_=p1, func=AF.Relu,
                bias=b1_sb[:, ht:ht + 1],
            )
        # L2: logits.T = W2.T @ h.T + b2  -> [1536, 512] bf16 as 12 ptiles
        # sg = sigmoid(-logits) in f32; log_1m = Ln(sg) batched afterwards so
        # the ScalarE LUT table loads twice per block, not per instruction.
        # et = exp(logits); softplus = Ln(1 + et) = -log_1m.  Exp/Ln/Relu all
        # live in ONE LUT table (natural_log_exp_and_others) -> no table
        # switches and no ordering constraints anywhere in the kernel.
        lT = lpool.tile([128, DT // 2, 2, NBS], f8)
        et = sgpool.tile([128, DT, NBS], bf, tag="et")
        for dt in range(DT):
            p2 = ps_l2.tile([128, NBS], f32)
            for c in range(HID // 256):
                nc.tensor.matmul(
                    p2,
                    lhsT=w2_sb[:, c, :, dt * 128:(dt + 1) * 128],
                    rhs=hT[:, c, :, :],
                    start=(c == 0), stop=(c == HID // 256 - 1),
                    perf_mode=mybir.MatmulPerfMode.DoubleRow,
                )
            # psum = 32 * (h @ W2); logits = psum/32 + b2  (on DVE: ScalarE is
            # the main-loop bottleneck, Vector has slack)
            nc.vector.tensor_scalar(
                lT[:, dt // 2, dt % 2, :], p2, 1.0 / 32.0, b2_sb[:, dt:dt + 1],
                op0=OP.mult, op1=OP.add,
            )
            nc.scalar.activation(
                out=et[:, dt, :], in_=p2, func=AF.Exp,
                scale=1.0 / 32.0, bias=b2_sb[:, dt:dt + 1],
            )
        sp = l1mpool.tile([128, DT, NBS], bf, tag="l1m")
        for dg in range(DT // 3):
            nc.scalar.activation(
                out=sp[:, dg * 3:(dg + 1) * 3, :],
                in_=et[:, dg * 3:(dg + 1) * 3, :], func=AF.Ln, bias=1.0,
            )
        pp = ps_p.tile([1, NBS], f32)
        for dt in range(DT):
            nc.tensor.matmul(
                pp, lhsT=ones_bf, rhs=sp[:, dt, :],
                start=(dt == 0), stop=(dt == DT - 1),
            )
        v_row = accpool.tile([1, NBS], f32, tag="v_row")
        nc.vector.tensor_scalar_mul(v_row, pp, -1.0)
        nc.gpsimd.partition_broadcast(v_bc[:, ncol], v_row)
        nc.vector.tensor_add(w_bc[:, ncol], v_bc[:, ncol], lw_bc[:, ncol])
        # scoring: G = x @ logits.T -> T_raw ; S = w + T_raw ; per-block top16
        for bt in range(BT):
            p3 = ps_sc.tile([128, NBS], f32)
            for c in range(D // 256):
                nc.tensor.matmul(
                    p3,
                    lhsT=xT_sb[:, c, :, bt * 128:(bt + 1) * 128],
                    rhs=lT[:, c, :, :],
                    start=(c == 0), stop=(c == D // 256 - 1),
                    perf_mode=mybir.MatmulPerfMode.DoubleRow,
                )
            # store S = w + G directly; scratch copy for the destructive top-k
            nc.vector.tensor_add(T_sb[:, bt, ncol], w_bc[:, ncol], p3)
            Sx = spool.tile([128, NBS], f32, tag="Sx")
            nc.vector.tensor_copy(Sx, T_sb[:, bt, ncol])
            cs = slice(nb * K, nb * K + 8)
            cs2 = slice(nb * K + 8, (nb + 1) * K)
            nc.vector.max(cand64[:, bt, cs], Sx)
            nc.vector.match_replace(
                out=Sx, in_to_replace=cand64[:, bt, cs], in_values=Sx,
                imm_value=NEG,
            )
            nc.vector.max(cand64[:, bt, cs2], Sx)

    # ---- merge per-block candidates -> local top-16 of S per row ----
    cand_dmas = []
    for bt in range(BT):
        c1 = cpool.tile([128, 8], f32, tag="c1")
        c2 = cpool.tile([128, 8], f32, tag="c2")
        nc.vector.max(c1, cand64[:, bt, :])
        nc.vector.match_replace(
            out=cand64[:, bt, :], in_to_replace=c1, in_values=cand64[:, bt, :],
            imm_value=NEG,
        )
        nc.vector.max(c2, cand64[:, bt, :])
        d1 = nc.sync.dma_start(out=cand_l[bt * 128:(bt + 1) * 128, 0:8], in_=c1)
        d2 = nc.sync.dma_start(out=cand_l[bt * 128:(bt + 1) * 128, 8:16], in_=c2)
        cand_dmas.extend([d1, d2])
        # local shifted max: -(local maxS + 16); exact rescale post-gather
        nc.vector.tensor_scalar(
            nmloc[:, bt:bt + 1], c1[:, 0:1], 16.0, -1.0, op0=OP.add, op1=OP.mult
        )

    nc.gpsimd.collective_compute(
        "AllGather", mybir.AluOpType.bypass,
        replica_groups=[list(range(NCORES))],
        ins=[cand_l.opt()], outs=[cand_a.opt()],
    )

    # ---- theta (16th of 128) and M (max + 16), identical on every core ----
    for bt in range(BT):
        la = cpool.tile([128, NCORES * K], f32, tag="la")
        nc.sync.dma_start(
            out=la.rearrange("p (c s) -> p c s", s=K),
            in_=cand_a[:, bt * 128:(bt + 1) * 128, :].rearrange("c p s -> p c s"),
        )
        t8a = cpool.tile([128, 8], f32, tag="t8a")
        t8b = cpool.tile([128, 8], f32, tag="t8b")
        nc.vector.max(t8a, la)
        nc.vector.tensor_scalar(
            negM[:, bt:bt + 1], t8a[:, 0:1], 16.0, -1.0, op0=OP.add, op1=OP.mult
        )
        nc.vector.match_replace(
            out=la, in_to_replace=t8a, in_values=la, imm_value=NEG
        )
        nc.vector.max(t8b, la)
        nc.vector.tensor_copy(TH[:, bt:bt + 1], t8b[:, 7:8])

    # ---- masked exp partial sums ----
    # Everything except the theta-select happens BEFORE/DURING the AllGather:
    # S = w + T_raw in place (bit-identical op to the candidate computation),
    # tv = T = S - lw, and E = exp(T - Mloc) against the LOCAL shifted max.
    # Post-gather only: esel = (S >= theta) * E, then partials are rescaled by
    # fac = exp(Mloc - M) so every core's sum is against the global M.
    for bt in range(BT):
        tv = epool.tile([128, SH], f32, tag="tv")
        ti = nc.vector.tensor_sub(tv, T_sb[:, bt, :], lw_bc)
        for dd in cand_dmas:
            _add_dep_helper(ti.ins, dd.ins,
                            reason="start collective before tail precompute")
        nc.scalar.activation(
            out=E_sb[:, bt, :], in_=tv, func=AF.Exp, bias=nmloc[:, bt:bt + 1]
        )
    for bt in range(BT):
        sel = epool.tile([128, SH], f32, tag="sel")
        nc.vector.scalar_tensor_tensor(
            out=sel, in0=T_sb[:, bt, :], scalar=TH[:, bt:bt + 1], in1=E_sb[:, bt, :],
            op0=OP.is_ge, op1=OP.mult,
            accum_out=Ssum[:, bt:bt + 1],
        )
        # fac = exp(Mloc - M) = exp((-nmloc) + negM ... ) computed from tiny APs
        nc.vector.tensor_sub(fac[:, bt:bt + 1], negM[:, bt:bt + 1],
                             nmloc[:, bt:bt + 1])
    nc.scalar.activation(out=fac, in_=fac, func=AF.Exp)
    nc.vector.tensor_mul(Ssum, Ssum, fac)

    nc.sync.dma_start(out=part_l.rearrange("(p t) -> p t", t=BT), in_=Ssum)
    nc.gpsimd.collective_compute(
        "AllReduce", mybir.AluOpType.add,
        replica_groups=[list(range(NCORES))],
        ins=[part_l.opt()], outs=[part_s.opt()],
    )
    nc.sync.dma_start(out=sum_sb, in_=part_s.rearrange("(p t) -> p t", t=BT))
    nc.scalar.activation(out=ln_sb, in_=sum_sb, func=AF.Ln)
    nc.vector.tensor_scalar_add(ln_sb, ln_sb, -math.log(float(K)))
    nc.vector.tensor_sub(out_sb, ln_sb, negM)  # + M
    nc.sync.dma_start(out=out_d.rearrange("(t p) -> p t", p=128), in_=out_sb)


def _build():
    f32 = mybir.dt.float32
    bf = mybir.dt.bfloat16
    f8 = mybir.dt.float8e4
    nc = bacc.Bacc(
        "TRN2", target_bir_lowering=False, debug=False, num_devices=NCORES
    )
    x_d = nc.dram_tensor("x", [128, D // 256, 2, B], f8, kind="ExternalInput").ap()
    z_d = nc.dram_tensor("z", [LAT, SH], bf, kind="ExternalInput").ap()
    w1_d = nc.dram_tensor("w1", [LAT, HID], bf, kind="ExternalInput").ap()
    w2_d = nc.dram_tensor("w2", [128, HID // 256, 2, D], f8, kind="ExternalInput").ap()
    b1_d = nc.dram_tensor("b1", [HID], f32, kind="ExternalInput").ap()
    b2_d = nc.dram_tensor("b2", [D], f32, kind="ExternalInput").ap()
    lw_d = nc.dram_tensor("lw", [SH], f32, kind="ExternalInput").ap()
    out_d = nc.dram_tensor("out", [B], f32, kind="ExternalOutput").ap()

    with tile.TileContext(nc) as tc:
        with ExitStack() as ctx:
            _body(tc, ctx, x_d, z_d, w1_d, w2_d, b1_d, b2_d, lw_d, out_d)
    nc.compile()
    return nc


def _get_built():
    global _BUILT
    if _BUILT is None:
        _BUILT = _build()
    return _BUILT


def make_in_maps(x, log_w, z, W1, b1, W2, b2):
    x = np.asarray(x, np.float32)
    log_w = np.asarray(log_w, np.float32)
    z = np.asarray(z, np.float32)
    # x.T packed for fp8 DoubleRow: [dp, c, j, b], d = 256c + 128j + dp
    xb = np.ascontiguousarray(
        x.astype(f8e3).T.reshape(6, 2, 128, B).transpose(2, 0, 1, 3))
    w1b = np.ascontiguousarray(np.asarray(W1, np.float32).astype(bf16))
    # (W2 * 32) packed for fp8 DoubleRow over K=HID: [kp, c, j, m]
    w2b = np.ascontiguousarray(
        (np.asarray(W2, np.float32) * 32.0).astype(f8e3)
        .reshape(2, 2, 128, D).transpose(2, 0, 1, 3))
    b1f = np.ascontiguousarray(np.asarray(b1, np.float32))
    b2f = np.ascontiguousarray(np.asarray(b2, np.float32))
    in_maps = []
    for c in range(NCORES):
        zs = np.ascontiguousarray(z[c * SH:(c + 1) * SH].astype(bf16).T)  # [LAT, SH]
        lws = np.ascontiguousarray(log_w[c * SH:(c + 1) * SH])
        in_maps.append(
            {"x": xb, "z": zs, "w1": w1b, "w2": w2b, "b1": b1f, "b2": b2f,
             "lw": lws}
        )
    return in_maps


def kernel(x, log_w, z, W1, b1, W2, b2, k, n_chunks):
    global LAST_RESULT
    assert int(k) == K, f"kernel compiled for k={K}, got {k}"
    nc = _get_built()
    in_maps = make_in_maps(x, log_w, z, W1, b1, W2, b2)
    res = run_bass_kernel_spmd(
        nc, in_maps, core_ids=list(range(NCORES)), trace=TRACE
    )
    LAST_RESULT = res
    return np.asarray(res.results[0]["out"], np.float32).reshape(B)


# revision 15
# speedup vs baseline: 1.0685x; 1.0637x over previous
"""Distributed Trainium2 kernel for nn_CategoricalDecoder (topk_masking).

Math (identical to the reference, algebraically simplified):
  logits = relu(z @ W1 + b1) @ W2 + b2                      # [NZ, D]
  log_prob_bins = x @ log_sig.T + (1-x) @ log_1m.T + log_w
                = x @ logits.T + v + log_w                  # log_sig - log_1m = logits
    where v[n] = sum_d log_1m[n, d],  log_1m = log_sigmoid(-logits)
  top-k selection by S = x@logits.T + v + log_w; the re-decode of the selected
  bins recomputes exactly T = S - log_w, so out = logsumexp(top16 T) - log 16.

Distribution: NZ=16384 sharded 8 ways (2048 bins/core); x and weights
replicated.  Per core: decode shard -> S,T slices -> local top-16 of S
(max8 x2) -> AllGather 16 candidates/core -> per-core identical theta
(16th largest of 128) and M (max + 16) -> masked exp partial sums ->
AllReduce -> log.  Every core computes the identical final [512] output.
"""

import math
from contextlib import ExitStack

import numpy as np
import ml_dtypes

import concourse.bass as bass
import concourse.mybir as mybir
import concourse.tile as tile
from concourse import bacc
from concourse.bass_utils import run_bass_kernel_spmd
from concourse import bass_isa
from concourse.bass import _add_dep_helper

bf16 = ml_dtypes.bfloat16
f8e3 = ml_dtypes.float8_e4m3

NCORES = 8
B, NZ, LAT, HID, D = 512, 16384, 64, 512, 1536
SH = NZ // NCORES      # 2048 bins per core
K = 16
NB = 4                 # column blocks per shard
NBS = SH // NB         # 512
BT = B // 128          # 4 row tiles
DT = D // 128          # 12
KT = HID // 128        # 4
NEG = -1.0e30

# module-level knobs for the local test harness (graded contract is kernel(**inputs))
TRACE = False
LAST_RESULT = None

_BUILT = None


def _patch_act_tables():
    """Steer every activation to the one LUT table holding all our funcs
    (Exp/Ln/Relu/Identity/Copy), so exactly one ACT_TABLE_LOAD is emitted.
    Only the chooser's view is patched; act_func_set_id indexing (and the
    table content walrus loads) is unchanged."""
    import concourse.bacc as _bm
    from concourse.hw_specs import get_activation_tables as _orig
    if getattr(_bm, "_act_tables_patched", False):
        return
    keep = "natural_log_exp_and_others"
    AF = mybir.ActivationFunctionType
    mine = {AF.Exp, AF.Ln, AF.Relu, AF.Identity, AF.Copy, AF.MemsetZero}

    def patched(arch):
        out = {}
        for name, funcs in _orig(arch).items():
            out[name] = funcs if name == keep else (set(funcs) - mine)
        return out

    _bm.get_activation_tables = patched
    _bm._act_tables_patched = True


def _body(tc, ctx, x_d, z_d, w1_d, w2_d, b1_d, b2_d, lw_d, out_d):
    nc = tc.nc
    f32 = mybir.dt.float32
    bf = mybir.dt.bfloat16
    f8 = mybir.dt.float8e4
    AF = mybir.ActivationFunctionType
    OP = mybir.AluOpType

    singles = ctx.enter_context(tc.tile_pool(name="singles", bufs=1))
    hpool = ctx.enter_context(tc.tile_pool(name="hpool", bufs=2))
    lpool = ctx.enter_context(tc.tile_pool(name="lpool", bufs=2))
    sgpool = ctx.enter_context(tc.tile_pool(name="sgpool", bufs=2))
    l1mpool = ctx.enter_context(tc.tile_pool(name="l1mpool", bufs=2))
    accpool = ctx.enter_context(tc.tile_pool(name="accpool", bufs=2))
    spool = ctx.enter_context(tc.tile_pool(name="spool", bufs=2))
    epool = ctx.enter_context(tc.tile_pool(name="epool", bufs=2))
    cpool = ctx.enter_context(tc.tile_pool(name="cpool", bufs=4))
    ps_l1 = ctx.enter_context(tc.tile_pool(name="ps_l1", bufs=1, space="PSUM"))
    ps_l2 = ctx.enter_context(tc.tile_pool(name="ps_l2", bufs=4, space="PSUM"))
    ps_sc = ctx.enter_context(tc.tile_pool(name="ps_sc", bufs=2, space="PSUM"))
    ps_p = ctx.enter_context(tc.tile_pool(name="ps_p", bufs=1, space="PSUM"))
    dram = ctx.enter_context(tc.tile_pool(name="dram", bufs=1, space="DRAM"))

    # persistent SBUF
    w1_sb = singles.tile([LAT, HID], bf)          # [64, 512]
    w2_sb = singles.tile([128, HID // 256, 2, D], f8)   # (kp, c, j, m), W2*32
    zT_sb = singles.tile([LAT, SH], bf)           # [64, 2048]
    xT_sb = singles.tile([128, D // 256, 2, B], f8)     # (dp, c, j, b)
    b1_sb = singles.tile([128, KT], f32)
    b2_sb = singles.tile([128, DT], f32)
    lw_bc = singles.tile([128, SH], f32)
    ones_bf = singles.tile([128, 1], bf)
    v_bc = singles.tile([128, SH], f32)           # v = sum_d log_1m, bcast on parts
    w_bc = singles.tile([128, SH], f32)           # w = v + log_w
    T_sb = singles.tile([128, BT, SH], f32)       # T_raw = G (x @ logits.T), no v
    cand64 = singles.tile([128, BT, 4 * K], f32)  # per-block top16 candidates
    TH = singles.tile([128, BT], f32)             # theta per (p, bt)
    negM = singles.tile([128, BT], f32)           # -(global maxS+16)
    nmloc = singles.tile([128, BT], f32)          # -(local maxS+16)
    fac = singles.tile([128, BT], f32)            # exp(Mloc - M)
    E_sb = singles.tile([128, BT, SH], bf)        # exp(T - Mloc), bf16
    Ssum = singles.tile([128, BT], f32)
    sum_sb = singles.tile([128, BT], f32)
    ln_sb = singles.tile([128, BT], f32)
    out_sb = singles.tile([128, BT], f32)

    cand_l = dram.tile([B, K], f32)
    cand_a = dram.tile([NCORES, B, K], f32)
    part_l = dram.tile([B], f32)
    part_s = dram.tile([B], f32)

    # ---- loads ----
    nc.vector.memset(ones_bf, 1.0)
    nc.sync.dma_start(out=w1_sb, in_=w1_d)
    nc.sync.dma_start(out=zT_sb, in_=z_d)
    nc.sync.dma_start(out=b1_sb, in_=b1_d.rearrange("(t p) -> p t", p=128))
    nc.sync.dma_start(out=b2_sb, in_=b2_d.rearrange("(t p) -> p t", p=128))
    nc.sync.dma_start(out=w2_sb, in_=w2_d)
    nc.sync.dma_start(out=xT_sb, in_=x_d)
    lw_bcast_ap = bass.AP(
        tensor=lw_d.tensor, offset=lw_d.offset, ap=[[0, 128]] + list(lw_d.ap)
    )
    nc.sync.dma_start(out=lw_bc, in_=lw_bcast_ap)

    # ---- decode + score, block by block ----
    for nb in range(NB):
        ncol = slice(nb * NBS, (nb + 1) * NBS)
        # L1: h.T = relu(W1.T @ z.T + b1)  -> [512, 512] bf16 as 4 ptiles
        hT = hpool.tile([128, HID // 256, 2, NBS], f8)
        for ht in range(KT):
            p1 = ps_l1.tile([128, NBS], f32)
            nc.tensor.matmul(
                p1, lhsT=w1_sb[:, ht * 128:(ht + 1) * 128], rhs=zT_sb[:, ncol],
                start=True, stop=True,
            )
            nc.scalar.activation(
                out=hT[:, ht // 2, ht % 2, :], in_=p1, func=AF.Relu,
                bias=b1_sb[:, ht:ht + 1],
            )
        # L2: logits.T = W2.T @ h.T + b2  -> [1536, 512] bf16 as 12 ptiles
        # sg = sigmoid(-logits) in f32; log_1m = Ln(sg) batched afterwards so
        # the ScalarE LUT table loads twice per block, not per instruction.
        # et = exp(logits); softplus = Ln(1 + et) = -log_1m.  Exp/Ln/Relu all
        # live in ONE LUT table (natural_log_exp_and_others) -> no table
        # switches and no ordering constraints anywhere in the kernel.
        lT = lpool.tile([128, DT // 2, 2, NBS], f8)
        et = sgpool.tile([128, DT, NBS], bf, tag="et")
        for dt in range(DT):
            p2 = ps_l2.tile([128, NBS], f32)
            for c in range(HID // 256):
                nc.tensor.matmul(
                    p2,
                    lhsT=w2_sb[:, c, :, dt * 128:(dt + 1) * 128],
                    rhs=hT[:, c, :, :],
                    start=(c == 0), stop=(c == HID // 256 - 1),
                    perf_mode=mybir.MatmulPerfMode.DoubleRow,
                )
            # psum = 32 * (h @ W2); logits = psum/32 + b2  (on DVE: ScalarE is
            # the main-loop bottleneck, Vector has slack)
            nc.vector.tensor_scalar(
                lT[:, dt // 2, dt % 2, :], p2, 1.0 / 32.0, b2_sb[:, dt:dt + 1],
                op0=OP.mult, op1=OP.add,
            )
            nc.scalar.activation(
                out=et[:, dt, :], in_=p2, func=AF.Exp,
                scale=1.0 / 32.0, bias=b2_sb[:, dt:dt + 1],
            )
        sp = l1mpool.tile([128, DT, NBS], bf, tag="l1m")
        for dg in range(DT // 3):
            nc.scalar.activation(
                out=sp[:, dg * 3:(dg + 1) * 3, :],
                in_=et[:, dg * 3:(dg + 1) * 3, :], func=AF.Ln, bias=1.0,
            )
        pp = ps_p.tile([1, NBS], f32)
        for dt in range(DT):
            nc.tensor.matmul(
                pp, lhsT=ones_bf, rhs=sp[:, dt, :],
                start=(dt == 0), stop=(dt == DT - 1),
            )
        v_row = accpool.tile([1, NBS], f32, tag="v_row")
        nc.vector.tensor_scalar_mul(v_row, pp, -1.0)
        nc.gpsimd.partition_broadcast(v_bc[:, ncol], v_row)
        nc.vector.tensor_add(w_bc[:, ncol], v_bc[:, ncol], lw_bc[:, ncol])
        # scoring: G = x @ logits.T -> T_raw ; S = w + T_raw ; per-block top16
        for bt in range(BT):
            p3 = ps_sc.tile([128, NBS], f32)
            for c in range(D // 256):
                nc.tensor.matmul(
                    p3,
                    lhsT=xT_sb[:, c, :, bt * 128:(bt + 1) * 128],
                    rhs=lT[:, c, :, :],
                    start=(c == 0), stop=(c == D // 256 - 1),
                    perf_mode=mybir.MatmulPerfMode.DoubleRow,
                )
            # store S = w + G directly; scratch copy for the destructive top-k
            nc.vector.tensor_add(T_sb[:, bt, ncol], w_bc[:, ncol], p3)
            Sx = spool.tile([128, NBS], f32, tag="Sx")
            nc.vector.tensor_copy(Sx, T_sb[:, bt, ncol])
            cs = slice(nb * K, nb * K + 8)
            cs2 = slice(nb * K + 8, (nb + 1) * K)
            nc.vector.max(cand64[:, bt, cs], Sx)
            nc.vector.match_replace(
                out=Sx, in_to_replace=cand64[:, bt, cs], in_values=Sx,
                imm_value=NEG,
            )
            nc.vector.max(cand64[:, bt, cs2], Sx)

    # ---- merge per-block candidates -> local top-16 of S per row ----
    cand_dmas = []
    for bt in range(BT):
        c1 = cpool.tile([128, 8], f32, tag="c1")
        c2 = cpool.tile([128, 8], f32, tag="c2")
        nc.vector.max(c1, cand64[:, bt, :])
        nc.vector.match_replace(
            out=cand64[:, bt, :], in_to_replace=c1, in_values=cand64[:, bt, :],
            imm_value=NEG,
        )
        nc.vector.max(c2, cand64[:, bt, :])
        d1 = nc.sync.dma_start(out=cand_l[bt * 128:(bt + 1) * 128, 0:8], in_=c1)
        d2 = nc.sync.dma_start(out=cand_l[bt * 128:(bt + 1) * 128, 8:16], in_=c2)
        cand_dmas.extend([d1, d2])
        # local shifted max: -(local maxS + 16); exact rescale post-gather
        nc.vector.tensor_scalar(
            nmloc[:, bt:bt + 1], c1[:, 0:1], 16.0, -1.0, op0=OP.add, op1=OP.mult
        )

    nc.gpsimd.collective_compute(
        "AllGather", mybir.AluOpType.bypass,
        replica_groups=[list(range(NCORES))],
        ins=[cand_l.opt()], outs=[cand_a.opt()],
    )

    # ---- theta (16th of 128) and M (max + 16), identical on every core ----
    for bt in range(BT):
        la = cpool.tile([128, NCORES * K], f32, tag="la")
        nc.sync.dma_start(
            out=la.rearrange("p (c s) -> p c s", s=K),
            in_=cand_a[:, bt * 128:(bt + 1) * 128, :].rearrange("c p s -> p c s"),
        )
        t8a = cpool.tile([128, 8], f32, tag="t8a")
        t8b = cpool.tile([128, 8], f32, tag="t8b")
        nc.vector.max(t8a, la)
        nc.vector.tensor_scalar(
            negM[:, bt:bt + 1], t8a[:, 0:1], 16.0, -1.0, op0=OP.add, op1=OP.mult
        )
        nc.vector.match_replace(
            out=la, in_to_replace=t8a, in_values=la, imm_value=NEG
        )
        nc.vector.max(t8b, la)
        nc.vector.tensor_copy(TH[:, bt:bt + 1], t8b[:, 7:8])

    # ---- masked exp partial sums ----
    # Everything except the theta-select happens BEFORE/DURING the AllGather:
    # S = w + T_raw in place (bit-identical op to the candidate computation),
    # tv = T = S - lw, and E = exp(T - Mloc) against the LOCAL shifted max.
    # Post-gather only: esel = (S >= theta) * E, then partials are rescaled by
    # fac = exp(Mloc - M) so every core's sum is against the global M.
    for bt in range(BT):
        tv = epool.tile([128, SH], f32, tag="tv")
        ti = nc.vector.tensor_sub(tv, T_sb[:, bt, :], lw_bc)
        for dd in cand_dmas:
            _add_dep_helper(ti.ins, dd.ins,
                            reason="start collective before tail precompute")
        nc.scalar.activation(
            out=E_sb[:, bt, :], in_=tv, func=AF.Exp, bias=nmloc[:, bt:bt + 1]
        )
    for bt in range(BT):
        sel = epool.tile([128, SH], f32, tag="sel")
        nc.vector.scalar_tensor_tensor(
            out=sel, in0=T_sb[:, bt, :], scalar=TH[:, bt:bt + 1], in1=E_sb[:, bt, :],
            op0=OP.is_ge, op1=OP.mult,
            accum_out=Ssum[:, bt:bt + 1],
        )
        # fac = exp(Mloc - M) = exp((-nmloc) + negM ... ) computed from tiny APs
        nc.vector.tensor_sub(fac[:, bt:bt + 1], negM[:, bt:bt + 1],
                             nmloc[:, bt:bt + 1])
    nc.scalar.activation(out=fac, in_=fac, func=AF.Exp)
    nc.vector.tensor_mul(Ssum, Ssum, fac)

    nc.sync.dma_start(out=part_l.rearrange("(p t) -> p t", t=BT), in_=Ssum)
    nc.gpsimd.collective_compute(
        "AllReduce", mybir.AluOpType.add,
        replica_groups=[list(range(NCORES))],
        ins=[part_l.opt()], outs=[part_s.opt()],
    )
    nc.sync.dma_start(out=sum_sb, in_=part_s.rearrange("(p t) -> p t", t=BT))
    nc.scalar.activation(out=ln_sb, in_=sum_sb, func=AF.Ln)
    nc.vector.tensor_scalar_add(ln_sb, ln_sb, -math.log(float(K)))
    nc.vector.tensor_sub(out_sb, ln_sb, negM)  # + M
    nc.sync.dma_start(out=out_d.rearrange("(t p) -> p t", p=128), in_=out_sb)


def _build():
    f32 = mybir.dt.float32
    bf = mybir.dt.bfloat16
    f8 = mybir.dt.float8e4
    _patch_act_tables()
    nc = bacc.Bacc(
        "TRN2", target_bir_lowering=False, debug=False, num_devices=NCORES
    )
    x_d = nc.dram_tensor("x", [128, D // 256, 2, B], f8, kind="ExternalInput").ap()
    z_d = nc.dram_tensor("z", [LAT, SH], bf, kind="ExternalInput").ap()
    w1_d = nc.dram_tensor("w1", [LAT, HID], bf, kind="ExternalInput").ap()
    w2_d = nc.dram_tensor("w2", [128, HID // 256, 2, D], f8, kind="ExternalInput").ap()
    b1_d = nc.dram_tensor("b1", [HID], f32, kind="ExternalInput").ap()
    b2_d = nc.dram_tensor("b2", [D], f32, kind="ExternalInput").ap()
    lw_d = nc.dram_tensor("lw", [SH], f32, kind="ExternalInput").ap()
    out_d = nc.dram_tensor("out", [B], f32, kind="ExternalOutput").ap()

    with tile.TileContext(nc) as tc:
        with ExitStack() as ctx:
            _body(tc, ctx, x_d, z_d, w1_d, w2_d, b1_d, b2_d, lw_d, out_d)
    nc.compile()
    return nc


def _get_built():
    global _BUILT
    if _BUILT is None:
        _BUILT = _build()
    return _BUILT


def make_in_maps(x, log_w, z, W1, b1, W2, b2):
    x = np.asarray(x, np.float32)
    log_w = np.asarray(log_w, np.float32)
    z = np.asarray(z, np.float32)
    # x.T packed for fp8 DoubleRow: [dp, c, j, b], d = 256c + 128j + dp
    xb = np.ascontiguousarray(
        x.astype(f8e3).T.reshape(6, 2, 128, B).transpose(2, 0, 1, 3))
    w1b = np.ascontiguousarray(np.asarray(W1, np.float32).astype(bf16))
    # (W2 * 32) packed for fp8 DoubleRow over K=HID: [kp, c, j, m]
    w2b = np.ascontiguousarray(
        (np.asarray(W2, np.float32) * 32.0).astype(f8e3)
        .reshape(2, 2, 128, D).transpose(2, 0, 1, 3))
    b1f = np.ascontiguousarray(np.asarray(b1, np.float32))
    b2f = np.ascontiguousarray(np.asarray(b2, np.float32))
    in_maps = []
    for c in range(NCORES):
        zs = np.ascontiguousarray(z[c * SH:(c + 1) * SH].astype(bf16).T)  # [LAT, SH]
        lws = np.ascontiguousarray(log_w[c * SH:(c + 1) * SH])
        in_maps.append(
            {"x": xb, "z": zs, "w1": w1b, "w2": w2b, "b1": b1f, "b2": b2f,
             "lw": lws}
        )
    return in_maps


def kernel(x, log_w, z, W1, b1, W2, b2, k, n_chunks):
    global LAST_RESULT
    assert int(k) == K, f"kernel compiled for k={K}, got {k}"
    nc = _get_built()
    in_maps = make_in_maps(x, log_w, z, W1, b1, W2, b2)
    res = run_bass_kernel_spmd(
        nc, in_maps, core_ids=list(range(NCORES)), trace=TRACE
    )
    LAST_RESULT = res
    return np.asarray(res.results[0]["out"], np.float32).reshape(B)


# revision 16
# speedup vs baseline: 1.1278x; 1.0555x over previous
"""Distributed Trainium2 kernel for nn_CategoricalDecoder (topk_masking).

Math (identical to the reference, algebraically simplified):
  logits = relu(z @ W1 + b1) @ W2 + b2                      # [NZ, D]
  log_prob_bins = x @ log_sig.T + (1-x) @ log_1m.T + log_w
                = x @ logits.T + v + log_w                  # log_sig - log_1m = logits
    where v[n] = sum_d log_1m[n, d],  log_1m = log_sigmoid(-logits)
  top-k selection by S = x@logits.T + v + log_w; the re-decode of the selected
  bins recomputes exactly T = S - log_w, so out = logsumexp(top16 T) - log 16.

Distribution: NZ=16384 sharded 8 ways (2048 bins/core); x and weights
replicated.  Per core: decode shard -> S,T slices -> local top-16 of S
(max8 x2) -> AllGather 16 candidates/core -> per-core identical theta
(16th largest of 128) and M (max + 16) -> masked exp partial sums ->
AllReduce -> log.  Every core computes the identical final [512] output.
"""

import math
from contextlib import ExitStack

import numpy as np
import ml_dtypes

import concourse.bass as bass
import concourse.mybir as mybir
import concourse.tile as tile
from concourse import bacc
from concourse.bass_utils import run_bass_kernel_spmd
from concourse import bass_isa
from concourse.bass import _add_dep_helper

bf16 = ml_dtypes.bfloat16
f8e3 = ml_dtypes.float8_e4m3

NCORES = 8
B, NZ, LAT, HID, D = 512, 16384, 64, 512, 1536
SH = NZ // NCORES      # 2048 bins per core
K = 16
NB = 4                 # column blocks per shard
NBS = SH // NB         # 512
BT = B // 128          # 4 row tiles
DT = D // 128          # 12
KT = HID // 128        # 4
NEG = -1.0e30

# module-level knobs for the local test harness (graded contract is kernel(**inputs))
TRACE = False
LAST_RESULT = None

_BUILT = None


def _patch_act_tables():
    """Steer every activation to the one LUT table holding all our funcs
    (Exp/Ln/Relu/Identity/Copy), so exactly one ACT_TABLE_LOAD is emitted.
    Only the chooser's view is patched; act_func_set_id indexing (and the
    table content walrus loads) is unchanged."""
    import concourse.bacc as _bm
    from concourse.hw_specs import get_activation_tables as _orig
    if getattr(_bm, "_act_tables_patched", False):
        return
    keep = "natural_log_exp_and_others"
    AF = mybir.ActivationFunctionType
    mine = {AF.Exp, AF.Ln, AF.Relu, AF.Identity, AF.Copy, AF.MemsetZero}

    def patched(arch):
        out = {}
        for name, funcs in _orig(arch).items():
            out[name] = funcs if name == keep else (set(funcs) - mine)
        return out

    _bm.get_activation_tables = patched
    _bm._act_tables_patched = True


def _body(tc, ctx, x_d, z_d, w1_d, w2_d, b1_d, b2_d, lw_d, out_d):
    nc = tc.nc
    f32 = mybir.dt.float32
    bf = mybir.dt.bfloat16
    f8 = mybir.dt.float8e4
    AF = mybir.ActivationFunctionType
    OP = mybir.AluOpType

    singles = ctx.enter_context(tc.tile_pool(name="singles", bufs=1))
    hpool = ctx.enter_context(tc.tile_pool(name="hpool", bufs=2))
    lpool = ctx.enter_context(tc.tile_pool(name="lpool", bufs=2))
    sgpool = ctx.enter_context(tc.tile_pool(name="sgpool", bufs=2))
    l1mpool = ctx.enter_context(tc.tile_pool(name="l1mpool", bufs=2))
    accpool = ctx.enter_context(tc.tile_pool(name="accpool", bufs=2))
    spool = ctx.enter_context(tc.tile_pool(name="spool", bufs=2))
    epool = ctx.enter_context(tc.tile_pool(name="epool", bufs=2))
    cpool = ctx.enter_context(tc.tile_pool(name="cpool", bufs=4))
    ps_l1 = ctx.enter_context(tc.tile_pool(name="ps_l1", bufs=1, space="PSUM"))
    ps_l2 = ctx.enter_context(tc.tile_pool(name="ps_l2", bufs=3, space="PSUM"))
    ps_sc = ctx.enter_context(tc.tile_pool(name="ps_sc", bufs=3, space="PSUM"))
    ps_p = ctx.enter_context(tc.tile_pool(name="ps_p", bufs=1, space="PSUM"))
    dram = ctx.enter_context(tc.tile_pool(name="dram", bufs=1, space="DRAM"))

    # persistent SBUF
    w1_sb = singles.tile([LAT, HID], bf)          # [64, 512]
    w2_sb = singles.tile([128, HID // 256, 2, D], f8)   # (kp, c, j, m), W2*32
    zT_sb = singles.tile([LAT, SH], bf)           # [64, 2048]
    xT_sb = singles.tile([128, D // 256, 2, B], f8)     # (dp, c, j, b)
    b1_sb = singles.tile([128, KT], f32)
    b2_sb = singles.tile([128, DT], f32)
    lw_bc = singles.tile([128, SH], f32)
    ones_bf = singles.tile([128, 1], bf)
    v_bc = singles.tile([128, SH], f32)           # v = sum_d log_1m, bcast on parts
    w_bc = singles.tile([128, SH], f32)           # w = v + log_w
    T_sb = singles.tile([128, BT, SH], f32)       # T_raw = G (x @ logits.T), no v
    cand64 = singles.tile([128, BT, 4 * K], f32)  # per-block top16 candidates
    TH = singles.tile([128, BT], f32)             # theta per (p, bt)
    negM = singles.tile([128, BT], f32)           # -(global maxS+16)
    nmloc = singles.tile([128, BT], f32)          # -(local maxS+16)
    fac = singles.tile([128, BT], f32)            # exp(Mloc - M)
    E_sb = singles.tile([128, BT, SH], bf)        # exp(T - Mloc), bf16
    Ssum = singles.tile([128, BT], f32)
    sum_sb = singles.tile([128, BT], f32)
    ln_sb = singles.tile([128, BT], f32)
    out_sb = singles.tile([128, BT], f32)

    cand_l = dram.tile([B, K], f32)
    cand_a = dram.tile([NCORES, B, K], f32)
    part_l = dram.tile([B], f32)
    part_s = dram.tile([B], f32)

    # ---- loads ----
    nc.vector.memset(ones_bf, 1.0)
    nc.sync.dma_start(out=w1_sb, in_=w1_d)
    nc.sync.dma_start(out=zT_sb, in_=z_d)
    nc.sync.dma_start(out=b1_sb, in_=b1_d.rearrange("(t p) -> p t", p=128))
    nc.sync.dma_start(out=b2_sb, in_=b2_d.rearrange("(t p) -> p t", p=128))
    nc.sync.dma_start(out=w2_sb, in_=w2_d)
    nc.sync.dma_start(out=xT_sb, in_=x_d)
    lw_bcast_ap = bass.AP(
        tensor=lw_d.tensor, offset=lw_d.offset, ap=[[0, 128]] + list(lw_d.ap)
    )
    nc.sync.dma_start(out=lw_bc, in_=lw_bcast_ap)

    # ---- decode + score, block by block ----
    for nb in range(NB):
        ncol = slice(nb * NBS, (nb + 1) * NBS)
        # L1: h.T = relu(W1.T @ z.T + b1)  -> [512, 512] bf16 as 4 ptiles
        hT = hpool.tile([128, HID // 256, 2, NBS], f8)
        for ht in range(KT):
            p1 = ps_l1.tile([128, NBS], f32)
            nc.tensor.matmul(
                p1, lhsT=w1_sb[:, ht * 128:(ht + 1) * 128], rhs=zT_sb[:, ncol],
                start=True, stop=True,
            )
            nc.scalar.activation(
                out=hT[:, ht // 2, ht % 2, :], in_=p1, func=AF.Relu,
                bias=b1_sb[:, ht:ht + 1],
            )
        # L2: logits.T = W2.T @ h.T + b2  -> [1536, 512] bf16 as 12 ptiles
        # sg = sigmoid(-logits) in f32; log_1m = Ln(sg) batched afterwards so
        # the ScalarE LUT table loads twice per block, not per instruction.
        # et = exp(logits); softplus = Ln(1 + et) = -log_1m.  Exp/Ln/Relu all
        # live in ONE LUT table (natural_log_exp_and_others) -> no table
        # switches and no ordering constraints anywhere in the kernel.
        lT = lpool.tile([128, DT // 2, 2, NBS], f8)
        et = sgpool.tile([128, DT, NBS], bf, tag="et")
        for dt in range(DT):
            p2 = ps_l2.tile([128, NBS], f32)
            for c in range(HID // 256):
                nc.tensor.matmul(
                    p2,
                    lhsT=w2_sb[:, c, :, dt * 128:(dt + 1) * 128],
                    rhs=hT[:, c, :, :],
                    start=(c == 0), stop=(c == HID // 256 - 1),
                    perf_mode=mybir.MatmulPerfMode.DoubleRow,
                )
            # psum = 32 * (h @ W2); logits = psum/32 + b2 -- split across the
            # two streaming engines to balance their load
            if dt % 3 == 2:
                nc.scalar.activation(
                    out=lT[:, dt // 2, dt % 2, :], in_=p2, func=AF.Identity,
                    scale=1.0 / 32.0, bias=b2_sb[:, dt:dt + 1],
                )
            else:
                nc.vector.tensor_scalar(
                    lT[:, dt // 2, dt % 2, :], p2, 1.0 / 32.0, b2_sb[:, dt:dt + 1],
                    op0=OP.mult, op1=OP.add,
                )
            nc.scalar.activation(
                out=et[:, dt, :], in_=p2, func=AF.Exp,
                scale=1.0 / 32.0, bias=b2_sb[:, dt:dt + 1],
            )
        sp = l1mpool.tile([128, DT, NBS], bf, tag="l1m")
        for dg in range(DT // 3):
            nc.scalar.activation(
                out=sp[:, dg * 3:(dg + 1) * 3, :],
                in_=et[:, dg * 3:(dg + 1) * 3, :], func=AF.Ln, bias=1.0,
            )
        pp = ps_p.tile([1, NBS], f32)
        for dt in range(DT):
            nc.tensor.matmul(
                pp, lhsT=ones_bf, rhs=sp[:, dt, :],
                start=(dt == 0), stop=(dt == DT - 1),
            )
        v_row = accpool.tile([1, NBS], f32, tag="v_row")
        nc.vector.tensor_scalar_mul(v_row, pp, -1.0)
        nc.gpsimd.partition_broadcast(v_bc[:, ncol], v_row)
        nc.vector.tensor_add(w_bc[:, ncol], v_bc[:, ncol], lw_bc[:, ncol])
        # scoring: G = x @ logits.T -> T_raw ; S = w + T_raw ; per-block top16
        for bt in range(BT):
            p3 = ps_sc.tile([128, NBS], f32)
            for c in range(D // 256):
                nc.tensor.matmul(
                    p3,
                    lhsT=xT_sb[:, c, :, bt * 128:(bt + 1) * 128],
                    rhs=lT[:, c, :, :],
                    start=(c == 0), stop=(c == D // 256 - 1),
                    perf_mode=mybir.MatmulPerfMode.DoubleRow,
                )
            # store S = w + G directly; scratch copy for the destructive top-k
            nc.vector.tensor_add(T_sb[:, bt, ncol], w_bc[:, ncol], p3)
            Sx = spool.tile([128, NBS], f32, tag="Sx")
            nc.vector.tensor_copy(Sx, T_sb[:, bt, ncol])
            cs = slice(nb * K, nb * K + 8)
            cs2 = slice(nb * K + 8, (nb + 1) * K)
            nc.vector.max(cand64[:, bt, cs], Sx)
            nc.vector.match_replace(
                out=Sx, in_to_replace=cand64[:, bt, cs], in_values=Sx,
                imm_value=NEG,
            )
            nc.vector.max(cand64[:, bt, cs2], Sx)

    # ---- merge per-block candidates -> local top-16 of S per row ----
    cand_dmas = []
    for bt in range(BT):
        c1 = cpool.tile([128, 8], f32, tag="c1")
        c2 = cpool.tile([128, 8], f32, tag="c2")
        nc.vector.max(c1, cand64[:, bt, :])
        nc.vector.match_replace(
            out=cand64[:, bt, :], in_to_replace=c1, in_values=cand64[:, bt, :],
            imm_value=NEG,
        )
        nc.vector.max(c2, cand64[:, bt, :])
        d1 = nc.sync.dma_start(out=cand_l[bt * 128:(bt + 1) * 128, 0:8], in_=c1)
        d2 = nc.sync.dma_start(out=cand_l[bt * 128:(bt + 1) * 128, 8:16], in_=c2)
        cand_dmas.extend([d1, d2])
        # local shifted max: -(local maxS + 16); exact rescale post-gather
        nc.vector.tensor_scalar(
            nmloc[:, bt:bt + 1], c1[:, 0:1], 16.0, -1.0, op0=OP.add, op1=OP.mult
        )

    nc.gpsimd.collective_compute(
        "AllGather", mybir.AluOpType.bypass,
        replica_groups=[list(range(NCORES))],
        ins=[cand_l.opt()], outs=[cand_a.opt()],
    )

    # ---- theta (16th of 128) and M (max + 16), identical on every core ----
    for bt in range(BT):
        la = cpool.tile([128, NCORES * K], f32, tag="la")
        nc.sync.dma_start(
            out=la.rearrange("p (c s) -> p c s", s=K),
            in_=cand_a[:, bt * 128:(bt + 1) * 128, :].rearrange("c p s -> p c s"),
        )
        t8a = cpool.tile([128, 8], f32, tag="t8a")
        t8b = cpool.tile([128, 8], f32, tag="t8b")
        nc.vector.max(t8a, la)
        nc.vector.tensor_scalar(
            negM[:, bt:bt + 1], t8a[:, 0:1], 16.0, -1.0, op0=OP.add, op1=OP.mult
        )
        nc.vector.match_replace(
            out=la, in_to_replace=t8a, in_values=la, imm_value=NEG
        )
        nc.vector.max(t8b, la)
        nc.vector.tensor_copy(TH[:, bt:bt + 1], t8b[:, 7:8])

    # ---- masked exp partial sums ----
    # Everything except the theta-select happens BEFORE/DURING the AllGather:
    # S = w + T_raw in place (bit-identical op to the candidate computation),
    # tv = T = S - lw, and E = exp(T - Mloc) against the LOCAL shifted max.
    # Post-gather only: esel = (S >= theta) * E, then partials are rescaled by
    # fac = exp(Mloc - M) so every core's sum is against the global M.
    for bt in range(BT):
        tv = epool.tile([128, SH], f32, tag="tv")
        ti = nc.vector.tensor_sub(tv, T_sb[:, bt, :], lw_bc)
        for dd in cand_dmas:
            _add_dep_helper(ti.ins, dd.ins,
                            reason="start collective before tail precompute")
        nc.scalar.activation(
            out=E_sb[:, bt, :], in_=tv, func=AF.Exp, bias=nmloc[:, bt:bt + 1]
        )
    for bt in range(BT):
        sel = epool.tile([128, SH], f32, tag="sel")
        nc.vector.scalar_tensor_tensor(
            out=sel, in0=T_sb[:, bt, :], scalar=TH[:, bt:bt + 1], in1=E_sb[:, bt, :],
            op0=OP.is_ge, op1=OP.mult,
            accum_out=Ssum[:, bt:bt + 1],
        )
        # fac = exp(Mloc - M) = exp((-nmloc) + negM ... ) computed from tiny APs
        nc.vector.tensor_sub(fac[:, bt:bt + 1], negM[:, bt:bt + 1],
                             nmloc[:, bt:bt + 1])
    nc.scalar.activation(out=fac, in_=fac, func=AF.Exp)
    nc.vector.tensor_mul(Ssum, Ssum, fac)

    nc.sync.dma_start(out=part_l.rearrange("(p t) -> p t", t=BT), in_=Ssum)
    nc.gpsimd.collective_compute(
        "AllReduce", mybir.AluOpType.add,
        replica_groups=[list(range(NCORES))],
        ins=[part_l.opt()], outs=[part_s.opt()],
    )
    nc.sync.dma_start(out=sum_sb, in_=part_s.rearrange("(p t) -> p t", t=BT))
    nc.scalar.activation(out=ln_sb, in_=sum_sb, func=AF.Ln)
    nc.vector.tensor_scalar_add(ln_sb, ln_sb, -math.log(float(K)))
    nc.vector.tensor_sub(out_sb, ln_sb, negM)  # + M
    nc.sync.dma_start(out=out_d.rearrange("(t p) -> p t", p=128), in_=out_sb)


def _build():
    f32 = mybir.dt.float32
    bf = mybir.dt.bfloat16
    f8 = mybir.dt.float8e4
    _patch_act_tables()
    nc = bacc.Bacc(
        "TRN2", target_bir_lowering=False, debug=False, num_devices=NCORES
    )
    x_d = nc.dram_tensor("x", [128, D // 256, 2, B], f8, kind="ExternalInput").ap()
    z_d = nc.dram_tensor("z", [LAT, SH], bf, kind="ExternalInput").ap()
    w1_d = nc.dram_tensor("w1", [LAT, HID], bf, kind="ExternalInput").ap()
    w2_d = nc.dram_tensor("w2", [128, HID // 256, 2, D], f8, kind="ExternalInput").ap()
    b1_d = nc.dram_tensor("b1", [HID], f32, kind="ExternalInput").ap()
    b2_d = nc.dram_tensor("b2", [D], f32, kind="ExternalInput").ap()
    lw_d = nc.dram_tensor("lw", [SH], f32, kind="ExternalInput").ap()
    out_d = nc.dram_tensor("out", [B], f32, kind="ExternalOutput").ap()

    with tile.TileContext(nc) as tc:
        with ExitStack() as ctx:
            _body(tc, ctx, x_d, z_d, w1_d, w2_d, b1_d, b2_d, lw_d, out_d)
    nc.compile()
    return nc


def _get_built():
    global _BUILT
    if _BUILT is None:
        _BUILT = _build()
    return _BUILT


def make_in_maps(x, log_w, z, W1, b1, W2, b2):
    x = np.asarray(x, np.float32)
    log_w = np.asarray(log_w, np.float32)
    z = np.asarray(z, np.float32)
    # x.T packed for fp8 DoubleRow: [dp, c, j, b], d = 256c + 128j + dp
    xb = np.ascontiguousarray(
        x.astype(f8e3).T.reshape(6, 2, 128, B).transpose(2, 0, 1, 3))
    w1b = np.ascontiguousarray(np.asarray(W1, np.float32).astype(bf16))
    # (W2 * 32) packed for fp8 DoubleRow over K=HID: [kp, c, j, m]
    w2b = np.ascontiguousarray(
        (np.asarray(W2, np.float32) * 32.0).astype(f8e3)
        .reshape(2, 2, 128, D).transpose(2, 0, 1, 3))
    b1f = np.ascontiguousarray(np.asarray(b1, np.float32))
    b2f = np.ascontiguousarray(np.asarray(b2, np.float32))
    in_maps = []
    for c in range(NCORES):
        zs = np.ascontiguousarray(z[c * SH:(c + 1) * SH].astype(bf16).T)  # [LAT, SH]
        lws = np.ascontiguousarray(log_w[c * SH:(c + 1) * SH])
        in_maps.append(
            {"x": xb, "z": zs, "w1": w1b, "w2": w2b, "b1": b1f, "b2": b2f,
             "lw": lws}
        )
    return in_maps


def kernel(x, log_w, z, W1, b1, W2, b2, k, n_chunks):
    global LAST_RESULT
    assert int(k) == K, f"kernel compiled for k={K}, got {k}"
    nc = _get_built()
    in_maps = make_in_maps(x, log_w, z, W1, b1, W2, b2)
    res = run_bass_kernel_spmd(
        nc, in_maps, core_ids=list(range(NCORES)), trace=TRACE
    )
    LAST_RESULT = res
    return np.asarray(res.results[0]["out"], np.float32).reshape(B)
